# revision 47
# baseline (speedup 1.0000x reference)
"""Adaptive max-pool-1d (ragged lengths) Trainium2 kernel — DMA-gather design.

Problem: x [32, 512, 4096] f32, length [32] i32 -> out [32, 512, 512] f32.
Per batch b with L = length[b]:
  L >= 512: PyTorch AdaptiveMaxPool1d over first L steps into 512 bins
  L < 512:  out[b,c,j] = x[b,c,j] if j < L else 0

Design (v2): the kernel is DMA-bound (cost model: one exclusive DMA resource
at 360 B/ns; SBUF ap_gather is charged max(input-span, indices) on GPSIMD at
~1.4 ns/elem, making any design that funnels the whole array through an SBUF
gather Pool-bound at >43us/core). Instead each batch's x is uploaded
TRANSPOSED as fp16 rows (row t = x[b, :, t], 1024 B) and the device gathers
the K window points of every output bin directly from HBM with
gpsimd.dma_gather — descriptor-count cost, not span cost. Gathered rows land
channel-contiguous in SBUF ([bin-slot, k, 512c], one bin per partition per
k-run), so the k-reduction is a chain of fp16 tensor_max ops that hit the
DVE 2x packed mode (0.54 ns/elem). Output is stored fp16 in [bin, channel]
layout; the host transposes/upcasts.

Work distribution: all 32 batches are cut into 128-bin chunks (4 per batch).
Chunks are classed by K (max adaptive window, <=9); classes whose size is
not a multiple of 8 borrow the largest chunks of the next class down
(processing a chunk at a higher K just duplicates gather points — harmless
under max), so every class splits evenly over the 8 cores. Every core runs
an IDENTICAL slot program (one aggregated slot per class) on its own chunks:
pure SPMD, no communication. L<512 batches use K=1 with out-of-range bins
pointing at a host-zeroed row (the copy+pad branch needs no control flow).

fp16 quantization of x bounds the relative error at 2^-11 ~ 4.9e-4, far
inside the 2e-2 gate; invalid bins are exact zeros.
"""

import sys

if "/opt/trn_rl_repo" not in sys.path:
    sys.path.insert(0, "/opt/trn_rl_repo")

import numpy as np

B, C, T, O = 32, 512, 4096, 512
NCORES = 8
CHUNK = 128                 # bins per chunk; 4 chunks per batch

_prog_cache = {}
_TRACE = False
_LAST = None                # last BassKernelResults (for test harness)


def _win(lb):
    """Window starts/ends [O] for one batch length (PyTorch adaptive pool)."""
    j = np.arange(O, dtype=np.int64)
    s = (j * lb) // O
    e = -((-(j + 1) * lb) // O)
    return s, e


def _exact_k(lb):
    if lb < O:
        return 1
    s, e = _win(lb)
    return int((e - s).max())


def _plan(L):
    """Split batches into per-core identical (bins, K) slots.

    Returns (slots, assign): slots is a tuple of (nbins_per_core, K);
    assign[core][slot] is the list of (batch, bin_start) chunks (length
    nbins_per_core // CHUNK) that core processes in that slot.
    """
    L = [int(v) for v in L]
    kb = [_exact_k(v) for v in L]
    chunks = sorted(
        ((kb[b], b, cs) for b in range(B) for cs in range(0, O, CHUNK)),
        key=lambda t: (-t[0], t[1], t[2]),
    )
    n = len(chunks)
    # effective K per chunk after promotion: walk classes from big K down;
    # if a class isn't a multiple of NCORES, promote the first chunks of the
    # next class (they are the largest-K ones there) into it.
    eff = [c[0] for c in chunks]
    i = 0
    while i < n:
        k = eff[i]
        j = i
        while j < n and eff[j] == k:
            j += 1
        rem = (j - i) % NCORES
        if rem:
            take = min(NCORES - rem, n - j)
            for t in range(j, j + take):
                eff[t] = k
            if take < NCORES - rem:
                # tail class too small to fill: pad with dummy chunks is not
                # supported; fold upward instead (promote this whole class
                # into the previous one). Rare; only matters for tiny B.
                raise ValueError("cannot balance chunk classes")
            j += take
        i = j
    # split each class into pieces of 1-2 chunks per core (targets ~1-1.5k
    # gather descriptors per piece so DMA holds stay fine-grained and each
    # piece's gather/accum/store pipelines against its neighbours)
    slots = []
    assign = [[] for _ in range(NCORES)]
    i = 0
    while i < n:
        k = eff[i]
        j = i
        while j < n and eff[j] == k:
            j += 1
        cls = chunks[i:j]
        per = len(cls) // NCORES
        percore = [[(cb, cs) for (_, cb, cs) in cls[c::NCORES]] for c in range(NCORES)]
        # wide-k classes merge into one piece per core (wider accum ops
        # amortize the ~150ns DVE op overhead; the >1024-desc gather is
        # k-split anyway), mid-k classes use 256-bin pieces, k==1 one piece
        step = per if k >= 7 or k == 1 else 2
        for p0 in range(0, per, step):
            p1 = min(p0 + step, per)
            slots.append(((p1 - p0) * CHUNK, k))
            for c in range(NCORES):
                assign[c].append(percore[c][p0:p1])
        i = j
    return tuple(slots), assign


def _build_program(slots, xt_rows):
    import concourse.bacc as bacc
    import concourse.mybir as mybir
    from concourse.tile import TileContext

    nc = bacc.Bacc()
    xt = nc.dram_tensor("xt", [xt_rows, C], mybir.dt.float16, kind="ExternalInput")
    coloff = []
    tot = 0
    for (nb, k) in slots:
        coloff.append(tot)
        tot += nb * k // 16
    idxall = nc.dram_tensor("idx", [128, tot], mybir.dt.int16, kind="ExternalInput")
    outs = []
    for si, (nb, k) in enumerate(slots):
        outs.append(
            nc.dram_tensor(f"out{si}", [nb, C], mybir.dt.float16,
                           kind="ExternalOutput")
        )

    # schedule: accum-heavy pieces (high DVE-work per DMA-byte, i.e. large
    # k) first so their chains drain while later gathers stream; shallow
    # accums at the end keep the tail short. The K=1 plain-load piece goes
    # LAST — its load has no deps (the queue runs it whenever there is
    # slack) and the final drain is just its store.
    order = sorted(
        range(len(slots)), key=lambda s: (-slots[s][1], -slots[s][0])
    )
    k1 = [s for s in order if slots[s][1] == 1]
    order = [s for s in order if slots[s][1] != 1] + k1
    with TileContext(nc) as tc:
        with tc.tile_pool(name="gp", bufs=6) as gpool, tc.tile_pool(
            name="ap", bufs=6
        ) as apool, tc.tile_pool(name="sc", bufs=4) as spool, tc.tile_pool(
            name="ip", bufs=1
        ) as ipool:
            # the first gather's indices ride a tiny priority DMA so its
            # descriptor generation starts ~0.5us earlier; the rest follow
            # in one bulk DMA
            iall = ipool.tile([128, tot], mybir.dt.int16, tag="idx")
            first = order[0]
            c1 = coloff[first] + slots[first][0] * slots[first][1] // 16
            nc.scalar.dma_start(out=iall[:, :c1], in_=idxall[:, :c1])
            nc.scalar.dma_start(out=iall[:, c1:], in_=idxall[:, c1:])
            # copy/zero-pad (K=1) pieces: the host stages these chunks' rows
            # (zero-padded) at the START of xt, so they are plain affine
            # loads with no gather and no idx dependency. Load now — it
            # fills the DMA pipe while the first gather's indices and
            # descriptors are still being set up — and store at the very
            # end, where the dep-free load means the drain is one store.
            k1tiles = {}
            for si in order:
                nb, k = slots[si]
                if k == 1:
                    gt = gpool.tile([128, (nb // 128) * C], mybir.dt.float16,
                                    tag=f"k1_{si}")
                    nc.sync.dma_start(
                        out=gt[:].rearrange("p (b c) -> p b c", c=C),
                        in_=xt[: nb, :].rearrange("(b p) c -> p b c", p=128),
                    )
                    k1tiles[si] = gt
            for oi, si in enumerate(order):
                nb, k = slots[si]
                npart = nb // 128          # bin slots per partition
                if k == 1:
                    ov = outs[si][:].rearrange("(b p) c -> p b c", p=128)
                    nc.sync.dma_start(
                        out=ov,
                        in_=k1tiles[si][:].rearrange("p (b c) -> p b c", c=C),
                    )
                    continue
                # hardware: one dma_gather handles at most 1024 indices —
                # split the piece's k-points into sub-gathers of <=1024
                kmax = max(1, 1024 // nb)
                nsub = -(-k // kmax)
                ksizes = [k // nsub + (1 if s < k % nsub else 0) for s in range(nsub)]
                col = coloff[si]
                work = []
                gts = []
                for ks in ksizes:
                    ni = nb * ks
                    gt = gpool.tile([128, npart * ks * C], mybir.dt.float16, tag="g")
                    nc.gpsimd.dma_gather(
                        gt[:].rearrange("p (i c) -> p i c", c=C), xt[:],
                        iall[:, col : col + ni // 16],
                        num_idxs=ni, num_idxs_reg=ni, elem_size=C,
                    )
                    col += ni // 16
                    gts.append(gt)
                    gv = gt[:].rearrange("p (b k c) -> p b k c", k=ks, c=C)
                    work.extend(gv[:, :, kk] for kk in range(ks))
                if k == 1:
                    st = gts[0]
                else:
                    at = apool.tile([128, npart * C], mybir.dt.float16, tag="a")
                    av = at[:].rearrange("p (b c) -> p b c", c=C)
                    # pairwise tree: same op count as a chain (k-1) but only
                    # ceil(log2 k) serial levels, so the piece drains fast
                    if k > 2:
                        sct = spool.tile(
                            [128, npart * (k // 2) * C], mybir.dt.float16, tag="s"
                        )
                        sv = sct[:].rearrange("p (b h c) -> p b h c", h=k // 2, c=C)
                        nxt = []
                        for h in range(k // 2):
                            nc.vector.tensor_max(
                                sv[:, :, h], work[2 * h], work[2 * h + 1]
                            )
                            nxt.append(sv[:, :, h])
                        if k % 2:
                            nxt.append(work[-1])
                        work = nxt
                    while len(work) > 2:
                        nxt = []
                        for h in range(len(work) // 2):
                            nc.vector.tensor_max(
                                work[2 * h], work[2 * h], work[2 * h + 1]
                            )
                            nxt.append(work[2 * h])
                        if len(work) % 2:
                            nxt.append(work[-1])
                        work = nxt
                    if len(work) == 2:
                        nc.vector.tensor_max(av, work[0], work[1])
                    else:
                        nc.vector.tensor_copy(av, work[0])
                    st = at
                # store: out row (b'*128 + p) <- acc[p, b']; alternate queues
                # so one piece's slow accum can't head-of-line block others
                ov = outs[si][:].rearrange("(b p) c -> p b c", p=128)
                steng = nc.sync if oi % 2 == 0 else nc.scalar
                steng.dma_start(
                    out=ov, in_=st[:].rearrange("p (b c) -> p b c", c=C)
                )
    nc.compile()
    return nc


def _chunk_indices(lb, k, cs, row_base, t0, zero_row):
    """Gather index cube [k, CHUNK] for bins [cs, cs+CHUNK) of one batch
    whose xT rows start at row_base (global row = row_base + t - t0)."""
    if lb < O:
        j = np.arange(cs, cs + CHUNK, dtype=np.int64)
        p = np.where(j < lb, row_base + j - t0, zero_row)[None, :]
        return np.broadcast_to(p, (k, CHUNK)).copy()
    s, e = _win(lb)
    s, e = s[cs : cs + CHUNK], e[cs : cs + CHUNK]
    kk = np.arange(k, dtype=np.int64)
    p = np.minimum(s[None, :] + kk[:, None], (e - 1)[None, :])  # [k, CHUNK]
    return p + (row_base - t0)


def _wrap_idx(tgt):
    """dma_gather wrapped layout: index m at [m % 16, m // 16], tiled x8."""
    n = tgt.shape[0]
    wrapped = tgt.reshape(n // 16, 16).T
    return np.ascontiguousarray(np.tile(wrapped, (8, 1)).astype(np.int16))


def kernel(x, length):
    global _LAST
    x = np.asarray(x)
    L = np.asarray(length).astype(np.int64).reshape(-1)
    x16 = x.astype(np.float16)

    slots, assign = _plan(L)

    # per-core xT row layout: for each chunk, the rows its windows span
    core_meta = []  # per core: per slot: list of (b, cs, row_base, t0, t1)
    core_rows = []
    # k==1 slots are staged at the START of xt (fixed program offset: they
    # are loaded with a plain DMA instead of a gather), one full CHUNK of
    # rows per chunk; gather slots follow.
    k1_rows = sum(nb for (nb, k) in slots if k == 1)
    for c in range(NCORES):
        rows = k1_rows
        k1_base = 0
        meta = []
        for si, (nb, k) in enumerate(slots):
            smeta = []
            for (b, cs) in assign[c][si]:
                lb = int(L[b])
                if k == 1:
                    smeta.append((b, cs, k1_base, cs, min(max(lb - cs, 0), CHUNK) + cs))
                    k1_base += CHUNK
                    continue
                if lb < O:
                    t0 = cs
                    t1 = max(t0, min(lb, cs + CHUNK))
                else:
                    s, e = _win(lb)
                    t0, t1 = int(s[cs]), int(e[cs + CHUNK - 1])
                smeta.append((b, cs, rows, t0, t1))
                rows += t1 - t0
            meta.append(smeta)
        core_meta.append(meta)
        core_rows.append(rows + 1)  # + zero row
    xt_rows = -(-max(core_rows) // 8) * 8

    key = (slots, xt_rows)
    if key not in _prog_cache:
        _prog_cache[key] = _build_program(slots, xt_rows)
    nc = _prog_cache[key]

    from concourse.bass_utils import run_bass_kernel_spmd

    in_maps = []
    for c in range(NCORES):
        m = {}
        xt = np.zeros((xt_rows, C), dtype=np.float16)
        zero_row = core_rows[c] - 1
        cols = []
        for si, (nb, k) in enumerate(slots):
            if k == 1:
                for (b, cs, row_base, t0, t1) in core_meta[c][si]:
                    xt[row_base : row_base + (t1 - t0)] = x16[b, :, t0:t1].T
                cols.append(np.zeros((128, nb // 16), np.int16))
                continue
            cubes = []
            for (b, cs, row_base, t0, t1) in core_meta[c][si]:
                lb = int(L[b])
                xt[row_base : row_base + (t1 - t0)] = x16[b, :, t0:t1].T
                cubes.append(_chunk_indices(lb, k, cs, row_base, t0, zero_row))
            # sub-gather split along k (mirror _build_program): each
            # sub-gather's flat order is chunk-major, then k, then bin
            kmax = max(1, 1024 // nb)
            nsub = -(-k // kmax)
            ksizes = [k // nsub + (1 if s < k % nsub else 0) for s in range(nsub)]
            k0 = 0
            for ks in ksizes:
                flat = np.concatenate([cb[k0 : k0 + ks] for cb in cubes])
                cols.append(_wrap_idx(flat.reshape(-1)))
                k0 += ks
        m["idx"] = np.ascontiguousarray(np.concatenate(cols, axis=1))
        m["xt"] = xt
        in_maps.append(m)

    res = None
    for attempt in range(3):
        try:
            res = run_bass_kernel_spmd(
                nc, in_maps, core_ids=list(range(NCORES)), trace=_TRACE
            )
            break
        except Exception:
            if attempt == 2:
                raise
    _LAST = res

    out = np.empty((B, C, O), dtype=np.float32)
    for c in range(NCORES):
        for si, (nb, k) in enumerate(slots):
            ob = np.asarray(res.results[c][f"out{si}"])  # [nb, C] fp16
            for ci, (b, cs, row_base, t0, t1) in enumerate(core_meta[c][si]):
                blk = ob[ci * CHUNK : (ci + 1) * CHUNK]
                out[b, :, cs : cs + CHUNK] = blk.astype(np.float32).T
    return out


# revision 49
# speedup vs baseline: 1.0135x; 1.0135x over previous
"""Adaptive max-pool-1d (ragged lengths) Trainium2 kernel — DMA-gather design.

Problem: x [32, 512, 4096] f32, length [32] i32 -> out [32, 512, 512] f32.
Per batch b with L = length[b]:
  L >= 512: PyTorch AdaptiveMaxPool1d over first L steps into 512 bins
  L < 512:  out[b,c,j] = x[b,c,j] if j < L else 0

Design (v2): the kernel is DMA-bound (cost model: one exclusive DMA resource
at 360 B/ns; SBUF ap_gather is charged max(input-span, indices) on GPSIMD at
~1.4 ns/elem, making any design that funnels the whole array through an SBUF
gather Pool-bound at >43us/core). Instead each batch's x is uploaded
TRANSPOSED as fp16 rows (row t = x[b, :, t], 1024 B) and the device gathers
the K window points of every output bin directly from HBM with
gpsimd.dma_gather — descriptor-count cost, not span cost. Gathered rows land
channel-contiguous in SBUF ([bin-slot, k, 512c], one bin per partition per
k-run), so the k-reduction is a chain of fp16 tensor_max ops that hit the
DVE 2x packed mode (0.54 ns/elem). Output is stored fp16 in [bin, channel]
layout; the host transposes/upcasts.

Work distribution: all 32 batches are cut into 128-bin chunks (4 per batch).
Chunks are classed by K (max adaptive window, <=9); classes whose size is
not a multiple of 8 borrow the largest chunks of the next class down
(processing a chunk at a higher K just duplicates gather points — harmless
under max), so every class splits evenly over the 8 cores. Every core runs
an IDENTICAL slot program (one aggregated slot per class) on its own chunks:
pure SPMD, no communication. L<512 batches use K=1 with out-of-range bins
pointing at a host-zeroed row (the copy+pad branch needs no control flow).

fp16 quantization of x bounds the relative error at 2^-11 ~ 4.9e-4, far
inside the 2e-2 gate; invalid bins are exact zeros.
"""

import sys

if "/opt/trn_rl_repo" not in sys.path:
    sys.path.insert(0, "/opt/trn_rl_repo")

import numpy as np

B, C, T, O = 32, 512, 4096, 512
NCORES = 8
CHUNK = 128                 # bins per chunk; 4 chunks per batch

_prog_cache = {}
_TRACE = False
_LAST = None                # last BassKernelResults (for test harness)


def _win(lb):
    """Window starts/ends [O] for one batch length (PyTorch adaptive pool)."""
    j = np.arange(O, dtype=np.int64)
    s = (j * lb) // O
    e = -((-(j + 1) * lb) // O)
    return s, e


def _exact_k(lb):
    if lb < O:
        return 1
    s, e = _win(lb)
    return int((e - s).max())


def _plan(L):
    """Split batches into per-core identical (bins, K) slots.

    Returns (slots, assign): slots is a tuple of (nbins_per_core, K);
    assign[core][slot] is the list of (batch, bin_start) chunks (length
    nbins_per_core // CHUNK) that core processes in that slot.
    """
    L = [int(v) for v in L]
    kb = [_exact_k(v) for v in L]
    chunks = sorted(
        ((kb[b], b, cs) for b in range(B) for cs in range(0, O, CHUNK)),
        key=lambda t: (-t[0], t[1], t[2]),
    )
    n = len(chunks)
    # effective K per chunk after promotion: walk classes from big K down;
    # if a class isn't a multiple of NCORES, promote the first chunks of the
    # next class (they are the largest-K ones there) into it.
    eff = [c[0] for c in chunks]
    i = 0
    while i < n:
        k = eff[i]
        j = i
        while j < n and eff[j] == k:
            j += 1
        rem = (j - i) % NCORES
        if rem:
            take = min(NCORES - rem, n - j)
            for t in range(j, j + take):
                eff[t] = k
            if take < NCORES - rem:
                # tail class too small to fill: pad with dummy chunks is not
                # supported; fold upward instead (promote this whole class
                # into the previous one). Rare; only matters for tiny B.
                raise ValueError("cannot balance chunk classes")
            j += take
        i = j
    # split each class into pieces of 1-2 chunks per core (targets ~1-1.5k
    # gather descriptors per piece so DMA holds stay fine-grained and each
    # piece's gather/accum/store pipelines against its neighbours)
    slots = []
    assign = [[] for _ in range(NCORES)]
    i = 0
    while i < n:
        k = eff[i]
        j = i
        while j < n and eff[j] == k:
            j += 1
        cls = chunks[i:j]
        per = len(cls) // NCORES
        percore = [[(cb, cs) for (_, cb, cs) in cls[c::NCORES]] for c in range(NCORES)]
        # wide-k classes merge into one piece per core (wider accum ops
        # amortize the ~150ns DVE op overhead; the >1024-desc gather is
        # k-split anyway), mid-k classes use 256-bin pieces, k==1 one piece
        step = per if k >= 7 or k == 1 else 2
        for p0 in range(0, per, step):
            p1 = min(p0 + step, per)
            slots.append(((p1 - p0) * CHUNK, k))
            for c in range(NCORES):
                assign[c].append(percore[c][p0:p1])
        i = j
    return tuple(slots), assign


def _build_program(slots, xt_rows):
    import concourse.bacc as bacc
    import concourse.mybir as mybir
    from concourse.tile import TileContext

    nc = bacc.Bacc()
    xt = nc.dram_tensor("xt", [xt_rows, C], mybir.dt.float16, kind="ExternalInput")
    coloff = []
    tot = 0
    for (nb, k) in slots:
        coloff.append(tot)
        tot += nb * k // 16
    idxall = nc.dram_tensor("idx", [128, tot], mybir.dt.int16, kind="ExternalInput")
    outs = []
    for si, (nb, k) in enumerate(slots):
        outs.append(
            nc.dram_tensor(f"out{si}", [nb, C], mybir.dt.float16,
                           kind="ExternalOutput")
        )

    # schedule: accum-heavy pieces (high DVE-work per DMA-byte, i.e. large
    # k) first so their chains drain while later gathers stream; shallow
    # accums at the end keep the tail short. The K=1 plain-load piece goes
    # LAST — its load has no deps (the queue runs it whenever there is
    # slack) and the final drain is just its store.
    order = sorted(
        range(len(slots)), key=lambda s: (-slots[s][1], -slots[s][0])
    )
    k1 = [s for s in order if slots[s][1] == 1]
    order = [s for s in order if slots[s][1] != 1] + k1
    with TileContext(nc) as tc:
        with tc.tile_pool(name="gp", bufs=6) as gpool, tc.tile_pool(
            name="ap", bufs=6
        ) as apool, tc.tile_pool(name="sc", bufs=4) as spool, tc.tile_pool(
            name="ip", bufs=1
        ) as ipool:
            # the first gather's indices ride a tiny priority DMA so its
            # descriptor generation starts ~0.5us earlier; the rest follow
            # in one bulk DMA
            iall = ipool.tile([128, tot], mybir.dt.int16, tag="idx")
            first = order[0]
            c1 = coloff[first] + slots[first][0] * slots[first][1] // 16
            nc.scalar.dma_start(out=iall[:, :c1], in_=idxall[:, :c1])
            nc.scalar.dma_start(out=iall[:, c1:], in_=idxall[:, c1:])
            # copy/zero-pad (K=1) pieces: the host stages these chunks' rows
            # (zero-padded, identity row order) at the START of xt, so the
            # whole piece is one dependency-free DRAM->DRAM copy. Issued
            # first, its transfer fills the otherwise-idle DMA window while
            # the first gather's indices/descriptors are in flight, and
            # nothing of it remains in the drain tail.
            # same queue as (and after) the idx DMAs so it cannot outrun the
            # first gather's index load on the exclusive DMA engine
            for si in order:
                nb, k = slots[si]
                if k == 1:
                    nc.scalar.dma_start(out=outs[si][:], in_=xt[:nb, :])
            for oi, si in enumerate(order):
                nb, k = slots[si]
                npart = nb // 128          # bin slots per partition
                if k == 1:
                    continue
                # hardware: one dma_gather handles at most 1024 indices —
                # split the piece's k-points into sub-gathers of <=1024
                kmax = max(1, 1024 // nb)
                nsub = -(-k // kmax)
                ksizes = [k // nsub + (1 if s < k % nsub else 0) for s in range(nsub)]
                col = coloff[si]
                work = []
                gts = []
                for ks in ksizes:
                    ni = nb * ks
                    gt = gpool.tile([128, npart * ks * C], mybir.dt.float16, tag="g")
                    nc.gpsimd.dma_gather(
                        gt[:].rearrange("p (i c) -> p i c", c=C), xt[:],
                        iall[:, col : col + ni // 16],
                        num_idxs=ni, num_idxs_reg=ni, elem_size=C,
                    )
                    col += ni // 16
                    gts.append(gt)
                    gv = gt[:].rearrange("p (b k c) -> p b k c", k=ks, c=C)
                    work.extend(gv[:, :, kk] for kk in range(ks))
                if k == 1:
                    st = gts[0]
                else:
                    at = apool.tile([128, npart * C], mybir.dt.float16, tag="a")
                    av = at[:].rearrange("p (b c) -> p b c", c=C)
                    # pairwise tree: same op count as a chain (k-1) but only
                    # ceil(log2 k) serial levels, so the piece drains fast
                    if k > 2:
                        sct = spool.tile(
                            [128, npart * (k // 2) * C], mybir.dt.float16, tag="s"
                        )
                        sv = sct[:].rearrange("p (b h c) -> p b h c", h=k // 2, c=C)
                        nxt = []
                        for h in range(k // 2):
                            nc.vector.tensor_max(
                                sv[:, :, h], work[2 * h], work[2 * h + 1]
                            )
                            nxt.append(sv[:, :, h])
                        if k % 2:
                            nxt.append(work[-1])
                        work = nxt
                    while len(work) > 2:
                        nxt = []
                        for h in range(len(work) // 2):
                            nc.vector.tensor_max(
                                work[2 * h], work[2 * h], work[2 * h + 1]
                            )
                            nxt.append(work[2 * h])
                        if len(work) % 2:
                            nxt.append(work[-1])
                        work = nxt
                    if len(work) == 2:
                        nc.vector.tensor_max(av, work[0], work[1])
                    else:
                        nc.vector.tensor_copy(av, work[0])
                    st = at
                # store: out row (b'*128 + p) <- acc[p, b']; alternate queues
                # so one piece's slow accum can't head-of-line block others
                ov = outs[si][:].rearrange("(b p) c -> p b c", p=128)
                steng = nc.sync if oi % 2 == 0 else nc.scalar
                steng.dma_start(
                    out=ov, in_=st[:].rearrange("p (b c) -> p b c", c=C)
                )
    nc.compile()
    return nc


def _chunk_indices(lb, k, cs, row_base, t0, zero_row):
    """Gather index cube [k, CHUNK] for bins [cs, cs+CHUNK) of one batch
    whose xT rows start at row_base (global row = row_base + t - t0)."""
    if lb < O:
        j = np.arange(cs, cs + CHUNK, dtype=np.int64)
        p = np.where(j < lb, row_base + j - t0, zero_row)[None, :]
        return np.broadcast_to(p, (k, CHUNK)).copy()
    s, e = _win(lb)
    s, e = s[cs : cs + CHUNK], e[cs : cs + CHUNK]
    kk = np.arange(k, dtype=np.int64)
    p = np.minimum(s[None, :] + kk[:, None], (e - 1)[None, :])  # [k, CHUNK]
    return p + (row_base - t0)


def _wrap_idx(tgt):
    """dma_gather wrapped layout: index m at [m % 16, m // 16], tiled x8."""
    n = tgt.shape[0]
    wrapped = tgt.reshape(n // 16, 16).T
    return np.ascontiguousarray(np.tile(wrapped, (8, 1)).astype(np.int16))


def kernel(x, length):
    global _LAST
    x = np.asarray(x)
    L = np.asarray(length).astype(np.int64).reshape(-1)
    x16 = x.astype(np.float16)

    slots, assign = _plan(L)

    # per-core xT row layout: for each chunk, the rows its windows span
    core_meta = []  # per core: per slot: list of (b, cs, row_base, t0, t1)
    core_rows = []
    # k==1 slots are staged at the START of xt (fixed program offset: they
    # are loaded with a plain DMA instead of a gather), one full CHUNK of
    # rows per chunk; gather slots follow.
    k1_rows = sum(nb for (nb, k) in slots if k == 1)
    for c in range(NCORES):
        rows = k1_rows
        k1_base = 0
        meta = []
        for si, (nb, k) in enumerate(slots):
            smeta = []
            for (b, cs) in assign[c][si]:
                lb = int(L[b])
                if k == 1:
                    smeta.append((b, cs, k1_base, cs, min(max(lb - cs, 0), CHUNK) + cs))
                    k1_base += CHUNK
                    continue
                if lb < O:
                    t0 = cs
                    t1 = max(t0, min(lb, cs + CHUNK))
                else:
                    s, e = _win(lb)
                    t0, t1 = int(s[cs]), int(e[cs + CHUNK - 1])
                smeta.append((b, cs, rows, t0, t1))
                rows += t1 - t0
            meta.append(smeta)
        core_meta.append(meta)
        core_rows.append(rows + 1)  # + zero row
    xt_rows = -(-max(core_rows) // 8) * 8

    key = (slots, xt_rows)
    if key not in _prog_cache:
        _prog_cache[key] = _build_program(slots, xt_rows)
    nc = _prog_cache[key]

    from concourse.bass_utils import run_bass_kernel_spmd

    in_maps = []
    for c in range(NCORES):
        m = {}
        xt = np.zeros((xt_rows, C), dtype=np.float16)
        zero_row = core_rows[c] - 1
        cols = []
        for si, (nb, k) in enumerate(slots):
            if k == 1:
                for (b, cs, row_base, t0, t1) in core_meta[c][si]:
                    xt[row_base : row_base + (t1 - t0)] = x16[b, :, t0:t1].T
                cols.append(np.zeros((128, nb // 16), np.int16))
                continue
            cubes = []
            for (b, cs, row_base, t0, t1) in core_meta[c][si]:
                lb = int(L[b])
                xt[row_base : row_base + (t1 - t0)] = x16[b, :, t0:t1].T
                cubes.append(_chunk_indices(lb, k, cs, row_base, t0, zero_row))
            # sub-gather split along k (mirror _build_program): each
            # sub-gather's flat order is chunk-major, then k, then bin
            kmax = max(1, 1024 // nb)
            nsub = -(-k // kmax)
            ksizes = [k // nsub + (1 if s < k % nsub else 0) for s in range(nsub)]
            k0 = 0
            for ks in ksizes:
                flat = np.concatenate([cb[k0 : k0 + ks] for cb in cubes])
                cols.append(_wrap_idx(flat.reshape(-1)))
                k0 += ks
        m["idx"] = np.ascontiguousarray(np.concatenate(cols, axis=1))
        m["xt"] = xt
        in_maps.append(m)

    res = None
    for attempt in range(3):
        try:
            res = run_bass_kernel_spmd(
                nc, in_maps, core_ids=list(range(NCORES)), trace=_TRACE
            )
            break
        except Exception:
            if attempt == 2:
                raise
    _LAST = res

    out = np.empty((B, C, O), dtype=np.float32)
    for c in range(NCORES):
        for si, (nb, k) in enumerate(slots):
            ob = np.asarray(res.results[c][f"out{si}"])  # [nb, C] fp16
            for ci, (b, cs, row_base, t0, t1) in enumerate(core_meta[c][si]):
                blk = ob[ci * CHUNK : (ci + 1) * CHUNK]
                out[b, :, cs : cs + CHUNK] = blk.astype(np.float32).T
    return out


# revision 51
# speedup vs baseline: 1.0146x; 1.0011x over previous
"""Adaptive max-pool-1d (ragged lengths) Trainium2 kernel — DMA-gather design.

Problem: x [32, 512, 4096] f32, length [32] i32 -> out [32, 512, 512] f32.
Per batch b with L = length[b]:
  L >= 512: PyTorch AdaptiveMaxPool1d over first L steps into 512 bins
  L < 512:  out[b,c,j] = x[b,c,j] if j < L else 0

Design (v2): the kernel is DMA-bound (cost model: one exclusive DMA resource
at 360 B/ns; SBUF ap_gather is charged max(input-span, indices) on GPSIMD at
~1.4 ns/elem, making any design that funnels the whole array through an SBUF
gather Pool-bound at >43us/core). Instead each batch's x is uploaded
TRANSPOSED as fp16 rows (row t = x[b, :, t], 1024 B) and the device gathers
the K window points of every output bin directly from HBM with
gpsimd.dma_gather — descriptor-count cost, not span cost. Gathered rows land
channel-contiguous in SBUF ([bin-slot, k, 512c], one bin per partition per
k-run), so the k-reduction is a chain of fp16 tensor_max ops that hit the
DVE 2x packed mode (0.54 ns/elem). Output is stored fp16 in [bin, channel]
layout; the host transposes/upcasts.

Work distribution: all 32 batches are cut into 128-bin chunks (4 per batch).
Chunks are classed by K (max adaptive window, <=9); classes whose size is
not a multiple of 8 borrow the largest chunks of the next class down
(processing a chunk at a higher K just duplicates gather points — harmless
under max), so every class splits evenly over the 8 cores. Every core runs
an IDENTICAL slot program (one aggregated slot per class) on its own chunks:
pure SPMD, no communication. L<512 batches use K=1 with out-of-range bins
pointing at a host-zeroed row (the copy+pad branch needs no control flow).

fp16 quantization of x bounds the relative error at 2^-11 ~ 4.9e-4, far
inside the 2e-2 gate; invalid bins are exact zeros.
"""

import sys

if "/opt/trn_rl_repo" not in sys.path:
    sys.path.insert(0, "/opt/trn_rl_repo")

import numpy as np

B, C, T, O = 32, 512, 4096, 512
NCORES = 8
CHUNK = 128                 # bins per chunk; 4 chunks per batch

_prog_cache = {}
_TRACE = False
_LAST = None                # last BassKernelResults (for test harness)


def _win(lb):
    """Window starts/ends [O] for one batch length (PyTorch adaptive pool)."""
    j = np.arange(O, dtype=np.int64)
    s = (j * lb) // O
    e = -((-(j + 1) * lb) // O)
    return s, e


def _exact_k(lb):
    if lb < O:
        return 1
    s, e = _win(lb)
    return int((e - s).max())


def _plan(L):
    """Split batches into per-core identical (bins, K) slots.

    Returns (slots, assign): slots is a tuple of (nbins_per_core, K);
    assign[core][slot] is the list of (batch, bin_start) chunks (length
    nbins_per_core // CHUNK) that core processes in that slot.
    """
    L = [int(v) for v in L]
    kb = [_exact_k(v) for v in L]
    chunks = sorted(
        ((kb[b], b, cs) for b in range(B) for cs in range(0, O, CHUNK)),
        key=lambda t: (-t[0], t[1], t[2]),
    )
    n = len(chunks)
    # effective K per chunk after promotion: walk classes from big K down;
    # if a class isn't a multiple of NCORES, promote the first chunks of the
    # next class (they are the largest-K ones there) into it.
    eff = [c[0] for c in chunks]
    i = 0
    while i < n:
        k = eff[i]
        j = i
        while j < n and eff[j] == k:
            j += 1
        rem = (j - i) % NCORES
        if rem:
            take = min(NCORES - rem, n - j)
            for t in range(j, j + take):
                eff[t] = k
            if take < NCORES - rem:
                # tail class too small to fill: pad with dummy chunks is not
                # supported; fold upward instead (promote this whole class
                # into the previous one). Rare; only matters for tiny B.
                raise ValueError("cannot balance chunk classes")
            j += take
        i = j
    # split each class into pieces of 1-2 chunks per core (targets ~1-1.5k
    # gather descriptors per piece so DMA holds stay fine-grained and each
    # piece's gather/accum/store pipelines against its neighbours)
    slots = []
    assign = [[] for _ in range(NCORES)]
    i = 0
    while i < n:
        k = eff[i]
        j = i
        while j < n and eff[j] == k:
            j += 1
        cls = chunks[i:j]
        per = len(cls) // NCORES
        percore = [[(cb, cs) for (_, cb, cs) in cls[c::NCORES]] for c in range(NCORES)]
        # wide-k classes merge into one piece per core (wider accum ops
        # amortize the ~150ns DVE op overhead; the >1024-desc gather is
        # k-split anyway), mid-k classes use 256-bin pieces, k==1 one piece
        step = per if k >= 7 or k == 1 else 2
        for p0 in range(0, per, step):
            p1 = min(p0 + step, per)
            slots.append(((p1 - p0) * CHUNK, k))
            for c in range(NCORES):
                assign[c].append(percore[c][p0:p1])
        i = j
    return tuple(slots), assign


def _build_program(slots, xt_rows):
    import concourse.bacc as bacc
    import concourse.mybir as mybir
    from concourse.tile import TileContext

    nc = bacc.Bacc()
    xt = nc.dram_tensor("xt", [xt_rows, C], mybir.dt.float16, kind="ExternalInput")
    coloff = []
    tot = 0
    for (nb, k) in slots:
        coloff.append(tot)
        tot += nb * k // 16
    idxall = nc.dram_tensor("idx", [128, tot], mybir.dt.int16, kind="ExternalInput")
    outs = []
    for si, (nb, k) in enumerate(slots):
        outs.append(
            nc.dram_tensor(f"out{si}", [nb, C], mybir.dt.float16,
                           kind="ExternalOutput")
        )

    # schedule: accum-heavy pieces (high DVE-work per DMA-byte, i.e. large
    # k) first so their chains drain while later gathers stream; shallow
    # accums at the end keep the tail short. The K=1 plain-load piece goes
    # LAST — its load has no deps (the queue runs it whenever there is
    # slack) and the final drain is just its store.
    order = sorted(
        range(len(slots)), key=lambda s: (-slots[s][1], -slots[s][0])
    )
    k1 = [s for s in order if slots[s][1] == 1]
    order = [s for s in order if slots[s][1] != 1] + k1
    with TileContext(nc) as tc:
        with tc.tile_pool(name="gp", bufs=6) as gpool, tc.tile_pool(
            name="ap", bufs=6
        ) as apool, tc.tile_pool(name="sc", bufs=4) as spool, tc.tile_pool(
            name="ip", bufs=1
        ) as ipool:
            # the first gather's indices ride a tiny priority DMA so its
            # descriptor generation starts ~0.5us earlier; the rest follow
            # in one bulk DMA
            iall = ipool.tile([128, tot], mybir.dt.int16, tag="idx")
            first = order[0]
            fnb, fk = slots[first]
            fkmax = max(1, 1024 // fnb)
            fns = -(-fk // fkmax)
            fk1 = fk // fns + (1 if 0 < fk % fns else 0)
            c1 = coloff[first] + fnb * fk1 // 16
            nc.scalar.dma_start(out=iall[:, :c1], in_=idxall[:, :c1])
            nc.scalar.dma_start(out=iall[:, c1:], in_=idxall[:, c1:])
            # copy/zero-pad (K=1) pieces: the host stages these chunks' rows
            # (zero-padded, identity row order) at the START of xt, so the
            # whole piece is one dependency-free DRAM->DRAM copy. Issued
            # first, its transfer fills the otherwise-idle DMA window while
            # the first gather's indices/descriptors are in flight, and
            # nothing of it remains in the drain tail.
            # same queue as (and after) the idx DMAs so it cannot outrun the
            # first gather's index load on the exclusive DMA engine
            for si in order:
                nb, k = slots[si]
                if k == 1:
                    nc.scalar.dma_start(out=outs[si][:], in_=xt[:nb, :])
            for oi, si in enumerate(order):
                nb, k = slots[si]
                npart = nb // 128          # bin slots per partition
                if k == 1:
                    continue
                # hardware: one dma_gather handles at most 1024 indices —
                # split the piece's k-points into sub-gathers of <=1024
                kmax = max(1, 1024 // nb)
                nsub = -(-k // kmax)
                ksizes = [k // nsub + (1 if s < k % nsub else 0) for s in range(nsub)]
                col = coloff[si]
                work = []
                gts = []
                for ks in ksizes:
                    ni = nb * ks
                    gt = gpool.tile([128, npart * ks * C], mybir.dt.float16, tag="g")
                    nc.gpsimd.dma_gather(
                        gt[:].rearrange("p (i c) -> p i c", c=C), xt[:],
                        iall[:, col : col + ni // 16],
                        num_idxs=ni, num_idxs_reg=ni, elem_size=C,
                    )
                    col += ni // 16
                    gts.append(gt)
                    gv = gt[:].rearrange("p (b k c) -> p b k c", k=ks, c=C)
                    work.extend(gv[:, :, kk] for kk in range(ks))
                if k == 1:
                    st = gts[0]
                else:
                    at = apool.tile([128, npart * C], mybir.dt.float16, tag="a")
                    av = at[:].rearrange("p (b c) -> p b c", c=C)
                    # pairwise tree: same op count as a chain (k-1) but only
                    # ceil(log2 k) serial levels, so the piece drains fast
                    if k > 2:
                        sct = spool.tile(
                            [128, npart * (k // 2) * C], mybir.dt.float16, tag="s"
                        )
                        sv = sct[:].rearrange("p (b h c) -> p b h c", h=k // 2, c=C)
                        nxt = []
                        for h in range(k // 2):
                            nc.vector.tensor_max(
                                sv[:, :, h], work[2 * h], work[2 * h + 1]
                            )
                            nxt.append(sv[:, :, h])
                        if k % 2:
                            nxt.append(work[-1])
                        work = nxt
                    while len(work) > 2:
                        nxt = []
                        for h in range(len(work) // 2):
                            nc.vector.tensor_max(
                                work[2 * h], work[2 * h], work[2 * h + 1]
                            )
                            nxt.append(work[2 * h])
                        if len(work) % 2:
                            nxt.append(work[-1])
                        work = nxt
                    if len(work) == 2:
                        nc.vector.tensor_max(av, work[0], work[1])
                    else:
                        nc.vector.tensor_copy(av, work[0])
                    st = at
                # store: out row (b'*128 + p) <- acc[p, b']; alternate queues
                # so one piece's slow accum can't head-of-line block others
                ov = outs[si][:].rearrange("(b p) c -> p b c", p=128)
                steng = nc.sync if oi % 2 == 0 else nc.scalar
                steng.dma_start(
                    out=ov, in_=st[:].rearrange("p (b c) -> p b c", c=C)
                )
    nc.compile()
    return nc


def _chunk_indices(lb, k, cs, row_base, t0, zero_row):
    """Gather index cube [k, CHUNK] for bins [cs, cs+CHUNK) of one batch
    whose xT rows start at row_base (global row = row_base + t - t0)."""
    if lb < O:
        j = np.arange(cs, cs + CHUNK, dtype=np.int64)
        p = np.where(j < lb, row_base + j - t0, zero_row)[None, :]
        return np.broadcast_to(p, (k, CHUNK)).copy()
    s, e = _win(lb)
    s, e = s[cs : cs + CHUNK], e[cs : cs + CHUNK]
    kk = np.arange(k, dtype=np.int64)
    p = np.minimum(s[None, :] + kk[:, None], (e - 1)[None, :])  # [k, CHUNK]
    return p + (row_base - t0)


def _wrap_idx(tgt):
    """dma_gather wrapped layout: index m at [m % 16, m // 16], tiled x8."""
    n = tgt.shape[0]
    wrapped = tgt.reshape(n // 16, 16).T
    return np.ascontiguousarray(np.tile(wrapped, (8, 1)).astype(np.int16))


def kernel(x, length):
    global _LAST
    x = np.asarray(x)
    L = np.asarray(length).astype(np.int64).reshape(-1)
    x16 = x.astype(np.float16)

    slots, assign = _plan(L)

    # per-core xT row layout: for each chunk, the rows its windows span
    core_meta = []  # per core: per slot: list of (b, cs, row_base, t0, t1)
    core_rows = []
    # k==1 slots are staged at the START of xt (fixed program offset: they
    # are loaded with a plain DMA instead of a gather), one full CHUNK of
    # rows per chunk; gather slots follow.
    k1_rows = sum(nb for (nb, k) in slots if k == 1)
    for c in range(NCORES):
        rows = k1_rows
        k1_base = 0
        meta = []
        for si, (nb, k) in enumerate(slots):
            smeta = []
            for (b, cs) in assign[c][si]:
                lb = int(L[b])
                if k == 1:
                    smeta.append((b, cs, k1_base, cs, min(max(lb - cs, 0), CHUNK) + cs))
                    k1_base += CHUNK
                    continue
                if lb < O:
                    t0 = cs
                    t1 = max(t0, min(lb, cs + CHUNK))
                else:
                    s, e = _win(lb)
                    t0, t1 = int(s[cs]), int(e[cs + CHUNK - 1])
                smeta.append((b, cs, rows, t0, t1))
                rows += t1 - t0
            meta.append(smeta)
        core_meta.append(meta)
        core_rows.append(rows + 1)  # + zero row
    xt_rows = -(-max(core_rows) // 8) * 8

    key = (slots, xt_rows)
    if key not in _prog_cache:
        _prog_cache[key] = _build_program(slots, xt_rows)
    nc = _prog_cache[key]

    from concourse.bass_utils import run_bass_kernel_spmd

    in_maps = []
    for c in range(NCORES):
        m = {}
        xt = np.zeros((xt_rows, C), dtype=np.float16)
        zero_row = core_rows[c] - 1
        cols = []
        for si, (nb, k) in enumerate(slots):
            if k == 1:
                for (b, cs, row_base, t0, t1) in core_meta[c][si]:
                    xt[row_base : row_base + (t1 - t0)] = x16[b, :, t0:t1].T
                cols.append(np.zeros((128, nb // 16), np.int16))
                continue
            cubes = []
            for (b, cs, row_base, t0, t1) in core_meta[c][si]:
                lb = int(L[b])
                xt[row_base : row_base + (t1 - t0)] = x16[b, :, t0:t1].T
                cubes.append(_chunk_indices(lb, k, cs, row_base, t0, zero_row))
            # sub-gather split along k (mirror _build_program): each
            # sub-gather's flat order is chunk-major, then k, then bin
            kmax = max(1, 1024 // nb)
            nsub = -(-k // kmax)
            ksizes = [k // nsub + (1 if s < k % nsub else 0) for s in range(nsub)]
            k0 = 0
            for ks in ksizes:
                flat = np.concatenate([cb[k0 : k0 + ks] for cb in cubes])
                cols.append(_wrap_idx(flat.reshape(-1)))
                k0 += ks
        m["idx"] = np.ascontiguousarray(np.concatenate(cols, axis=1))
        m["xt"] = xt
        in_maps.append(m)

    res = None
    for attempt in range(3):
        try:
            res = run_bass_kernel_spmd(
                nc, in_maps, core_ids=list(range(NCORES)), trace=_TRACE
            )
            break
        except Exception:
            if attempt == 2:
                raise
    _LAST = res

    out = np.empty((B, C, O), dtype=np.float32)
    for c in range(NCORES):
        for si, (nb, k) in enumerate(slots):
            ob = np.asarray(res.results[c][f"out{si}"])  # [nb, C] fp16
            for ci, (b, cs, row_base, t0, t1) in enumerate(core_meta[c][si]):
                blk = ob[ci * CHUNK : (ci + 1) * CHUNK]
                out[b, :, cs : cs + CHUNK] = blk.astype(np.float32).T
    return out
